# revision 15
# baseline (speedup 1.0000x reference)
"""Trainium2 Bass kernel for nn_BasicLSTM (B=64, T=512, D=512, U=1024).

Data-parallel over batch across 8 NeuronCores (8 sequences per core, the
recurrence fully local per core — no cross-core communication).

Two structural ideas over the straightforward schedule:

1. x@Wx+b is hoisted out of the recurrence into a pre-pass that runs at
   M=128 (full PE stationary width, 4x the efficiency of in-loop M=8
   matmuls), streams the result xz to an internal DRAM scratch, and the
   per-step loop re-injects xz[t] into PSUM with one cheap K=9
   identity-matmul per N-tile (the 9th row adds the bias).

2. The LSTM cell update runs in unit-major layout: the sigmoid outputs
   (batch-major [gates x 8, 512] per bank) are PE-transposed into
   sT[128 units, gate*64 + chunk*8 + batch] so every DVE/ACT tail op is a
   [128, 32] tensor (all 128 lanes busy) instead of [8, 512] (8 lanes).
   The tail output hT IS the lhsT layout the next step's h-matmul needs,
   so the old transpose-h + copy stage disappears.  g-gate columns are
   host-prescaled by 2 so one sigmoid pass covers all gates
   (tanh(x) = 2*sigmoid(2x) - 1).

Matmul operands are bf16 (fp32 PSUM accumulation); c stays fp32.
"""

import os
import numpy as np
import ml_dtypes

_DIS = set(os.environ.get("KDIS", "").split(",")) - {""}

B, T, D, U = 64, 512, 512, 1024
NCORES = 8
BL = B // NCORES          # 8 sequences per core
NKX = D // 128            # 4 x K-chunks
NKH = U // 128            # 8 h K-chunks
NT = 512                  # N-tile width (one PSUM bank)
GOFF = (0, 32, 64, 96)    # PSUM partition offset per col-group (i,f,o,g)


def _build_nc(t_steps=T):
    import concourse.bass as bass
    import concourse.mybir as mybir

    f32, bf16 = mybir.dt.float32, mybir.dt.bfloat16
    AF = mybir.ActivationFunctionType
    ALU = mybir.AluOpType

    assert t_steps % 16 == 0
    MT = t_steps * BL // 128       # pre-pass M tiles (rows = (t, b) pairs)
    NPP = MT * 8                   # pre-pass tiles (x 8 N-tiles)
    TB = t_steps * BL

    nc = bass.Bass(num_devices=NCORES)
    wq = nc.declare_dram_parameter("wq", [1537, 4096], bf16, isOutput=False)
    xq = nc.declare_dram_parameter("xq", [NKX, 128, t_steps, BL], bf16, isOutput=False)
    idb_d = nc.declare_dram_parameter("idb", [9, 128], bf16, isOutput=False)
    idr_d = nc.declare_dram_parameter("idr", [104, 8], bf16, isOutput=False)
    out_d = nc.declare_dram_parameter("out", [128, 64], f32, isOutput=True)
    xz_d = nc.dram_tensor("xz_d", [t_steps, BL, 4096], bf16, kind="Internal")

    from contextlib import ExitStack
    ctx = ExitStack()
    sb = lambda shape, dt, name: ctx.enter_context(nc.sbuf_tensor(name, shape, dt))
    ps = lambda shape, dt, name: ctx.enter_context(nc.psum_tensor(name, shape, dt))
    sem = lambda name: ctx.enter_context(nc.semaphore(name))

    with ctx:
        wh_sb = sb([128, NKH * 4096], bf16, "wh_sb")
        wx_sb = sb([128, NKX * 4096], bf16, "wx_sb")
        x_sb = sb([128, NKX * TB], bf16, "x_sb")
        xzb = [sb([9, 4096], bf16, f"xzb{i}") for i in range(2)]
        # s_sb[dbuf][bank]; dbuf-0 pair doubles as the pre-pass stage buffers
        s_sb = [[sb([128, NT], bf16, f"s{d}{k}") for k in range(2)] for d in range(2)]
        idb_sb = sb([9, 128], bf16, "idb_sb")
        idr_sb = sb([104, 8], bf16, "idr_sb")
        gm_sb = sb([128, 64], bf16, "gm_sb")
        t1_sb = sb([128, 64], bf16, "t1_sb")
        c1_sb = sb([128, 64], f32, "c1_sb")
        cT = sb([128, 64], f32, "cT")
        tc_sb = sb([128, 64], bf16, "tc_sb")
        hT = [sb([128, 64], bf16, f"hT{i}") for i in range(2)]
        hlastT = sb([128, 64], f32, "hlastT")
        fgo_sb = [sb([32, 3 * NT], bf16, f"fgo{i}") for i in range(2)]

        zb = [ps([128, NT], f32, f"zb{i}") for i in range(4)]
        # sT[dbuf][bank]: per-bank PSUM tensors, full-bank sized so no two
        # share a physical bank (PE-W one bank while DVE-R the other is only
        # legal across distinct banks)
        sT = [[ps([128, 1024], bf16, f"sT{d}{k}") for k in range(2)] for d in range(2)]

        dma_sem = sem("dma_sem")
        ppld = sem("ppld")
        ppmm = sem("ppmm")
        ppcp = sem("ppcp")
        ppdma = [sem("ppdma0"), sem("ppdma1")]
        xz_sem = [sem("xz0"), sem("xz1")]
        op_sem = sem("op_sem")
        mm_sem = sem("mm_sem")
        sig_sem = sem("sig_sem")
        tr_sem = sem("tr_sem")
        oc_sem = sem("oc_sem")
        c_sem = sem("c_sem")
        tanh_sem = sem("tanh_sem")
        h_sem = sem("h_sem")

        with nc.Block() as block:

            @block.sync
            def _(sync):
                # wx + x first: the pre-pass only needs these 8 DMAs
                for kc in range(NKX):
                    sync.dma_start(
                        out=wx_sb[:, kc * 4096:(kc + 1) * 4096],
                        in_=wq[kc * 128:(kc + 1) * 128, :],
                    ).then_inc(ppld, 16)
                for kc in range(NKX):
                    sync.dma_start(
                        out=x_sb[:, kc * TB:(kc + 1) * TB],
                        in_=xq[kc],
                    ).then_inc(ppld, 16)
                for kc in range(NKH):
                    sync.dma_start(
                        out=wh_sb[:, kc * 4096:(kc + 1) * 4096],
                        in_=wq[(NKX + kc) * 128:(NKX + kc + 1) * 128, :],
                    ).then_inc(dma_sem, 16)
                sync.dma_start(out=idb_sb[:, :], in_=idb_d[:, :]).then_inc(dma_sem, 16)
                sync.dma_start(out=idr_sb[:, :], in_=idr_d[:, :]).then_inc(dma_sem, 16)
                for i in range(2):
                    sync.dma_start(out=xzb[i][8:9, :], in_=wq[1536:1537, :]).then_inc(dma_sem, 16)

                # pre-pass: stream xz tiles to DRAM scratch
                for k in range(NPP):
                    mt, nn = divmod(k, 8)
                    sync.wait_ge(ppcp, k + 1)
                    sync.dma_start(
                        out=xz_d[mt * 16:(mt + 1) * 16, :, nn * NT:(nn + 1) * NT],
                        in_=s_sb[0][k % 2][:, :],
                    ).then_inc(ppdma[k % 2], 16)

                # per-step xz streaming
                for t in range(t_steps):
                    if t < 2:
                        sync.wait_ge(ppdma[0], 16 * ((NPP + 1) // 2))
                        sync.wait_ge(ppdma[1], 16 * (NPP // 2))
                    else:
                        sync.wait_ge(op_sem, t - 1)
                    sync.dma_start(
                        out=xzb[t % 2][0:8, :], in_=xz_d[t],
                    ).then_inc(xz_sem[t % 2], 16)

                sync.wait_ge(h_sem, 2 * t_steps)
                sync.dma_start(out=out_d[:, :], in_=hlastT[:, :]).then_inc(dma_sem, 16)

            @block.tensor
            def _(tensor):
                tensor.wait_ge(ppld, 16 * 8)
                # ---- xz pre-pass: M=128 tiles, ping-pong over zb[0]/zb[1]
                for k in range(NPP):
                    mt, nn = divmod(k, 8)
                    if k >= 2:
                        tensor.wait_ge(ppcp, k - 1)
                    for kc in range(NKX):
                        ins = tensor.matmul(
                            zb[k % 2][:, :],
                            x_sb[:, kc * TB + mt * 128: kc * TB + (mt + 1) * 128],
                            wx_sb[:, kc * 4096 + nn * NT: kc * 4096 + (nn + 1) * NT],
                            start=(kc == 0), stop=(kc == NKX - 1),
                            skip_group_check=True,
                        )
                    ins.then_inc(ppmm, 1)
                tensor.wait_ge(dma_sem, 16 * (NKH + 4))
                tensor.wait_ge(ppcp, NPP)

                def openers(t, t0=False):
                    # z(t) := xz[t] + b via K=9 identity matmul (row 8 = ones)
                    tensor.wait_ge(xz_sem[t % 2], 16 * (t // 2 + 1))
                    zA, zB = zb[(t % 2) * 2], zb[(t % 2) * 2 + 1]
                    for bk, z in ((0, zA), (1, zB)):
                        for cg in range(4):
                            ncol = (4 * bk + cg) * NT
                            if cg == 0:
                                # M=128: writes xz+b into rows 0:8 and ZEROS
                                # into all other partitions (clears the bank
                                # so later start=False matmuls accumulate)
                                ins = tensor.matmul(
                                    z[:, :],
                                    idb_sb[:, :],
                                    xzb[t % 2][:, ncol:ncol + NT],
                                    start=True, stop=False,
                                    skip_group_check=True,
                                )
                            else:
                                ins = tensor.matmul(
                                    z[GOFF[cg]:GOFF[cg] + BL, :],
                                    idb_sb[:, 0:8],
                                    xzb[t % 2][:, ncol:ncol + NT],
                                    start=False, stop=(t0 and cg == 3),
                                    tile_position=(0, GOFF[cg]),
                                    skip_group_check=True,
                                )
                            if t0 and cg == 3:
                                ins.then_inc(mm_sem, 1)
                    if not t0:
                        # op_sem == t means openers(t) done; engine order makes
                        # that imply openers(t-1), (t-2), ... done as well.
                        # (t=0 carries no inc: its last matmul holds mm_sem.)
                        ins.then_inc(op_sem, 1)
                    elif t > 0:
                        # noh-bisect mode: op_sem rides the bank-A cg3 opener
                        zA2 = zb[(t % 2) * 2]
                        tensor.matmul(
                            zA2[GOFF[1]:GOFF[1] + BL, :],
                            idb_sb[:, 0:8],
                            xzb[t % 2][:, NT:2 * NT],
                            start=False, stop=False,
                            tile_position=(0, GOFF[1]),
                            skip_group_check=True,
                        ).then_inc(op_sem, 1)

                def h_mms(t):
                    zA, zB = zb[(t % 2) * 2], zb[(t % 2) * 2 + 1]
                    rd = (t + 1) % 2   # hT written at step t-1
                    for half in range(2):
                        tensor.wait_ge(h_sem, 2 * (t - 1) + half + 1)
                        for j in range(half * 4, half * 4 + 4):
                            for bk, z in ((0, zA), (1, zB)):
                                for cg in range(4):
                                    ncol = (4 * bk + cg) * NT
                                    last = (j == 7 and cg == 3)
                                    ins = tensor.matmul(
                                        z[GOFF[cg]:GOFF[cg] + BL, :],
                                        hT[rd][:, j * BL:(j + 1) * BL],
                                        wh_sb[:, j * 4096 + ncol: j * 4096 + ncol + NT],
                                        start=False, stop=last,
                                        tile_position=(0, GOFF[cg]),
                                        skip_group_check=True,
                                    )
                                    if last:
                                        ins.then_inc(mm_sem, 1)

                def transposes(t):
                    # per-bank sT cols: i 0:32 | f 32:64 | o 64:96 | g 96:128.
                    # Transposes must all run in PE row-group 0: any transpose
                    # pinned to row-group 1-3 aborts on HW when K=128 matmuls
                    # are in flight.  The f/g/o gate rows (partitions 32/64/96)
                    # are DVE-moved to partition 0 (fgo_sb) first; the i-gate
                    # transposes straight out of s_sb.
                    for bk in range(2):
                        tensor.wait_ge(sig_sem, 2 * t + bk + 1)
                        for u4 in range(4):
                            tensor.matmul(
                                sT[t % 2][bk][:, u4 * 8:(u4 + 1) * 8],
                                s_sb[t % 2][bk][0:BL, u4 * 128:(u4 + 1) * 128],
                                idr_sb[0:BL, :],
                                start=True, stop=True,
                                is_transpose=True,
                                tile_position=(0, 0),
                                skip_group_check=True,
                            )
                        tensor.wait_ge(oc_sem, 2 * t + bk + 1)
                        for gi, dst in ((0, 32), (1, 96), (2, 64)):   # f, g, o
                            for u4 in range(4):
                                ins = tensor.matmul(
                                    sT[t % 2][bk][:, dst + u4 * 8: dst + (u4 + 1) * 8],
                                    fgo_sb[bk][0:BL, gi * NT + u4 * 128: gi * NT + (u4 + 1) * 128],
                                    idr_sb[0:BL, :],
                                    start=True, stop=True,
                                    is_transpose=True,
                                    tile_position=(0, 0),
                                    skip_group_check=True,
                                )
                        ins.then_inc(tr_sem, 1)

                if "noh" in _DIS:
                    openers(0, t0=True)
                    openers(1, t0=True)
                    for t in range(t_steps):
                        transposes(t)
                        if t + 2 < t_steps:
                            openers(t + 2, t0=True)
                else:
                    openers(0, t0=True)
                    openers(1)
                    for t in range(t_steps):
                        if t > 0:
                            h_mms(t)
                        transposes(t)
                        if t + 2 < t_steps:
                            openers(t + 2)

            @block.scalar
            def _(scalar):
                # pre-pass PSUM -> SBUF stage copies
                for k in range(NPP):
                    scalar.wait_ge(ppmm, k + 1)
                    if k >= 2:
                        scalar.wait_ge(ppdma[k % 2], 16 * (k // 2))
                    nc.scalar.copy(s_sb[0][k % 2][:, :], zb[k % 2][:, :]).then_inc(ppcp, 1)

                for t in range(t_steps):
                    for bk in range(2):
                        scalar.wait_ge(mm_sem, 2 * t + bk + 1)
                        if t >= 2:
                            scalar.wait_ge(tr_sem, 2 * (t - 2) + bk + 1)
                            scalar.wait_ge(oc_sem, 2 * (t - 2) + bk + 1)
                        nc.scalar.activation(
                            s_sb[t % 2][bk][:, :], zb[(t % 2) * 2 + bk][:, :],
                            AF.Sigmoid,
                        ).then_inc(sig_sem, 1)
                    for bk in range(2):
                        scalar.wait_ge(c_sem, 2 * t + bk + 1)
                        if t >= 1:
                            scalar.wait_ge(h_sem, 2 * (t - 1) + bk + 1)
                        nc.scalar.activation(
                            tc_sb[:, bk * 32:(bk + 1) * 32], cT[:, bk * 32:(bk + 1) * 32],
                            AF.Tanh,
                        ).then_inc(tanh_sem, 1)

            @block.vector
            def _(vector):
                ALU = mybir.AluOpType
                nc.vector.memset(cT[:, :], 0.0)
                vector.drain()
                for t in range(t_steps):
                    for bk in range(2):
                        # move f/g/o gate rows (partitions 32/64/96) down to
                        # partition 0 for the row-group-0 transposes
                        vector.wait_ge(sig_sem, 2 * t + bk + 1)
                        if t >= 1:
                            vector.wait_ge(tr_sem, 2 * (t - 1) + bk + 1)
                        for gi in range(3):
                            ins = nc.vector.tensor_scalar(
                                fgo_sb[bk][:, gi * NT:(gi + 1) * NT],
                                s_sb[t % 2][bk][32 * (gi + 1):32 * (gi + 2), :],
                                1.0, 0.0, ALU.mult, ALU.add,
                            )
                        ins.then_inc(oc_sem, 1)
                    for bk in range(2):
                        sTt = sT[t % 2][bk]
                        lo, hi = bk * 32, (bk + 1) * 32
                        vector.wait_ge(tr_sem, 2 * t + bk + 1)
                        if "notail" in _DIS:
                            nc.vector.memset(cT[:, lo:hi], 0.25)
                            vector.drain()
                            nc.vector.nop().then_inc(c_sem, 1)
                            continue
                        # gm = tanh(zg) = 2*sigmoid(2 zg) - 1
                        nc.vector.tensor_scalar(
                            gm_sb[:, lo:hi], sTt[:, 96:128],
                            2.0, -1.0, ALU.mult, ALU.add,
                        )
                        nc.vector.tensor_mul(
                            c1_sb[:, lo:hi], sTt[:, 32:64], cT[:, lo:hi])
                        vector.drain()
                        nc.vector.tensor_mul(
                            t1_sb[:, lo:hi], gm_sb[:, lo:hi], sTt[:, 0:32])
                        vector.drain()
                        nc.vector.tensor_add(
                            cT[:, lo:hi], c1_sb[:, lo:hi], t1_sb[:, lo:hi],
                        ).then_inc(c_sem, 1)
                        vector.drain()
                    for bk in range(2):
                        sTt = sT[t % 2][bk]
                        lo, hi = bk * 32, (bk + 1) * 32
                        vector.wait_ge(tanh_sem, 2 * t + bk + 1)
                        dst = hT[t % 2] if t < t_steps - 1 else hlastT
                        if "notail" in _DIS:
                            nc.vector.memset(dst[:, lo:hi], 0.01)
                            vector.drain()
                            nc.vector.nop().then_inc(h_sem, 1)
                            continue
                        nc.vector.tensor_mul(
                            dst[:, lo:hi], sTt[:, 64:96], tc_sb[:, lo:hi],
                        ).then_inc(h_sem, 1)

    return nc


def _prep_w(Wx, Wh, b):
    """[Wx; Wh; b] rows, columns permuted to per-bank [i|f|g|o] blocks,
    g-gate columns pre-scaled by 2 (tanh(x) = 2*sigmoid(2x) - 1)."""
    bf16 = ml_dtypes.bfloat16
    Wfull = np.concatenate([Wx, Wh, b[None, :]], axis=0).astype(np.float32)
    cols = []
    for bank in range(2):
        u0, u1 = bank * NT, (bank + 1) * NT
        cols.append(np.arange(0 * U + u0, 0 * U + u1))       # i
        cols.append(np.arange(1 * U + u0, 1 * U + u1))       # f
        cols.append(np.arange(2 * U + u0, 2 * U + u1))       # g
        cols.append(np.arange(3 * U + u0, 3 * U + u1))       # o
    perm = np.concatenate(cols)
    Wp = Wfull[:, perm].copy()
    for bank in range(2):
        g0 = bank * 4 * NT + 2 * NT
        Wp[:, g0:g0 + NT] *= 2.0
    return np.ascontiguousarray(Wp).astype(bf16)


def _make_in_maps(x, Wx, Wh, b):
    bf16 = ml_dtypes.bfloat16
    t_steps = x.shape[1]
    Wp = _prep_w(Wx, Wh, b)
    idb = np.zeros((9, 128), dtype=bf16)
    for i in range(8):
        idb[i, i] = 1.0
        idb[8, i] = 1.0
    idr = np.zeros((104, 8), dtype=bf16)
    for off in GOFF:
        for i in range(8):
            idr[off + i, i] = 1.0
    in_maps = []
    for core in range(NCORES):
        xs = x[core * BL:(core + 1) * BL].astype(np.float32)      # [BL, T, D]
        xt = np.ascontiguousarray(np.transpose(xs, (2, 1, 0)))    # [D, T, BL]
        xt = xt.reshape(NKX, 128, t_steps, BL)
        in_maps.append({
            "wq": Wp,
            "xq": np.ascontiguousarray(xt).astype(bf16),
            "idb": idb,
            "idr": idr,
        })
    return in_maps


def _decode_out(o):
    """[128, 64] unit-major -> [BL, U] batch-major: h[b, u] = o[u%128, (u//128)*8+b]."""
    o = np.asarray(o, dtype=np.float32).reshape(128, 8, 8)    # [p, chunk, b]
    return np.ascontiguousarray(np.transpose(o, (2, 1, 0))).reshape(8, U)


def kernel(x, Wx, Wh, b):
    x = np.asarray(x, dtype=np.float32)
    Wx = np.asarray(Wx, dtype=np.float32)
    Wh = np.asarray(Wh, dtype=np.float32)
    b = np.asarray(b, dtype=np.float32)
    t_steps = x.shape[1]

    in_maps = _make_in_maps(x, Wx, Wh, b)
    nc = _build_nc(t_steps)

    from concourse.bass_utils import run_bass_kernel_spmd
    core_ids = list(range(NCORES))
    res = run_bass_kernel_spmd(nc, in_maps, core_ids,
                               trace=bool(globals().get("TRACE", False)))
    globals()["LAST_EXEC_NS"] = res.exec_time_ns

    h_parts = [_decode_out(res.results[i]["out"]) for i in core_ids]
    return np.concatenate(h_parts, axis=0)
